# revision 10
# baseline (speedup 1.0000x reference)
"""Trainium2 Bass kernel for a 5-layer LSTM classifier (PaperLSTMClassifier).

Model: B=1024, T=1024, H=64, L=5 layers, V=32 vocab, variable lengths.
Strategy: data-parallel over 8 NeuronCores (128 batch columns each).

Device-side design:
  - State feature-major [H, B] in lanes 0-63 of per-layer tiles hh[l] (Hs)
    and st[l] (Dc). Gate matmuls split into two K=64 base-0 matmuls
    accumulating in PSUM (Wx^T.T @ input + Wh^T.T @ Hs_l), reading producer
    tiles in place.
  - Re-parameterization: Hs = h/2, Dc = c/2; all gates Sigmoid-only
    (tanh(x) = 2 sig(2x) - 1 folded into host-scaled weights):
       [f; i]  = sig(mm + [b_f; b_i])      lanes: f 0-63, i 64-127
       [o; g'] = sig(mm + [b_o; 2 b_g])    lanes: o 0-63, g' 64-127
       fD = f * Dc                          lanes 0-63   (gpsimd)
       iG = (g' - 1/2) * i                  lanes 64-127 (DVE)
       Dc' = fD + iG  via PE pair-sum matmul -> PSUM lanes 0-63
       u   = sig(4 Dc')                     lanes 0-63   (ACT)
       Hs' = (u - 1/2) * o                  lanes 0-63   (DVE)
    Host folds every scale into the weights; math is exact.
  - Software-pipelined emission: slot s emits layer l's step for timestep
    t = s - l, ordered [all gate matmuls][all sig chains][all pair-sums]
    [all state updates][h4 tap]. Each layer's matmuls depend only on the
    previous slot, so engines never head-of-line block. Reference
    sequential semantics are preserved exactly (layer l still consumes
    h_{l-1} of the same timestep, computed one slot earlier; WAR deps keep
    single-buffered state correct).
  - No length masking: columns evolve independently; the h4 tap (PE
    transpose -> PSUM -> ACT copy -> SBUF stage) records h4^T for t = s-4
    every slot; one DMA per iteration writes the stage to a DRAM ring
    Y[nit+1, BL, U, HD]; one indirect DMA gathers row (t=len[b]-1, b) at
    the end; head matmul on device.
"""

import numpy as np

B, T, HD, L, V = 1024, 1024, 64, 5, 32
NCORES = 8
BL = B // NCORES          # 128 batch columns per core
CAT = 2 * HD              # 128
RAMP = L - 1              # 4 ramp-in and 4 ramp-out slots
U = 12                    # steady slots per For_i iteration
NIT = (T - RAMP) // U     # 85 iterations (1020 steady slots)
assert RAMP + NIT * U == T

_COMPILED = {}


def _build(u, nit):
    from contextlib import ExitStack

    import concourse.bass as bass
    import concourse.tile as tile
    from concourse import bacc, mybir
    from concourse.alu_op_type import AluOpType
    from concourse.bass import ds

    f32 = mybir.dt.float32
    i32 = mybir.dt.int32
    SIG = mybir.ActivationFunctionType.Sigmoid
    SUB = AluOpType.subtract
    MUL = AluOpType.mult
    ramp = RAMP

    nc = bacc.Bacc("TRN2", target_bir_lowering=False, debug=False)

    wmm_d = nc.dram_tensor("wmm", [L, 2, 2, 64, 128], f32, kind="ExternalInput")
    bias_d = nc.dram_tensor("biasv", [128, 2 * L], f32, kind="ExternalInput")
    hs0_d = nc.dram_tensor("hs0", [L, 64, BL], f32, kind="ExternalInput")
    exs_d = nc.dram_tensor("exs", [ramp, HD, BL], f32, kind="ExternalInput")
    exm_d = nc.dram_tensor("exm", [nit, HD, u * BL], f32, kind="ExternalInput")
    gidx_d = nc.dram_tensor("gidx", [BL, 1], i32, kind="ExternalInput")
    headw_d = nc.dram_tensor("headw", [HD, 1], f32, kind="ExternalInput")
    perm_d = nc.dram_tensor("permm", [2, 128, 128], f32, kind="ExternalInput")
    out_d = nc.dram_tensor("out", [BL, 1], f32, kind="ExternalOutput")
    # slot s taps t = s-4: steady slot (i, j) -> y[i, :, j, :]; the 4 tail
    # taps (t = 1020..1023) -> y[nit, :, 0..3, :]
    y_d = nc.dram_tensor("yring", [nit + 1, BL, u, HD], f32)

    with tile.TileContext(nc) as tc, ExitStack() as ctx:
        const = ctx.enter_context(tc.tile_pool(name="const", bufs=1))
        psg = ctx.enter_context(
            tc.tile_pool(name="psg", bufs=5, space=bass.MemorySpace.PSUM)
        )
        pst = ctx.enter_context(
            tc.tile_pool(name="pst", bufs=2, space=bass.MemorySpace.PSUM)
        )
        gates = ctx.enter_context(tc.tile_pool(name="gates", bufs=7))
        prods = ctx.enter_context(tc.tile_pool(name="prods", bufs=7))
        upool = ctx.enter_context(tc.tile_pool(name="upool", bufs=7))
        stages = ctx.enter_context(tc.tile_pool(name="stages", bufs=2))
        rstage = ctx.enter_context(tc.tile_pool(name="rstage", bufs=2))

        # --- constants ---
        wx = const.tile([64, L, 2, 128], f32, tag="wx")
        wh = const.tile([64, L, 2, 128], f32, tag="wh")
        for l in range(L):
            for g in range(2):
                nc.sync.dma_start(wx[:, l, g, :], wmm_d[l, g, 0])
                nc.sync.dma_start(wh[:, l, g, :], wmm_d[l, g, 1])
        bsb = const.tile([128, 2 * L], f32, tag="bsb")
        nc.sync.dma_start(bsb[:], bias_d[:])
        ident = const.tile([128, 128], f32, tag="ident")
        nc.sync.dma_start(ident[:], perm_d[0])
        p2sb = const.tile([128, 128], f32, tag="p2sb")
        nc.sync.dma_start(p2sb[:], perm_d[1])
        gidx_sb = const.tile([BL, 1], i32, tag="gidx")
        nc.sync.dma_start(gidx_sb[:], gidx_d[:])
        headw_sb = const.tile([HD, 1], f32, tag="headw")
        nc.sync.dma_start(headw_sb[:], headw_d[:])

        # --- persistent state (lanes 0-63) ---
        hh = []
        st = []
        for l in range(L):
            h_t = const.tile([128, BL], f32, tag=f"hh{l}")
            nc.vector.memset(h_t[:], 0.0)
            nc.sync.dma_start(h_t[0:64, :], hs0_d[l])
            hh.append(h_t)
            s_t = const.tile([64, BL], f32, tag=f"st{l}")
            nc.vector.memset(s_t[:], 0.0)
            st.append(s_t)

        def slot(layers, ex_rhs, y_stage_ap):
            """Emit one pipeline slot. layers: active layer list. ex_rhs:
            layer-0 rhs AP or None. y_stage_ap: tap destination or None."""
            pss = {}
            for l in layers:
                ps = psg.tile([128, 384], f32, tag="ps", bufs=5)
                pss[l] = ps
                for g in range(2):
                    cols = slice(128 * g, 128 * (g + 1))
                    rhs = ex_rhs if l == 0 else hh[l - 1][0:64, :]
                    nc.tensor.matmul(
                        ps[:, cols], wx[:, l, g, :], rhs,
                        start=True, stop=False,
                    )
                    nc.tensor.matmul(
                        ps[:, cols], wh[:, l, g, :], hh[l][0:64, :],
                        start=False, stop=True,
                    )
            gt = {}
            for l in layers:
                ps = pss[l]
                sbfi = gates.tile([128, BL], f32, tag="sbfi")  # [f; i]
                sbog = gates.tile([128, BL], f32, tag="sbog")  # [o; g']
                nc.scalar.activation(
                    sbfi[:], ps[:, 0:128], SIG, bias=bsb[:, 2 * l : 2 * l + 1]
                )
                nc.scalar.activation(
                    sbog[:], ps[:, 128:256], SIG,
                    bias=bsb[:, 2 * l + 1 : 2 * l + 2],
                )
                pr = prods.tile([128, BL], f32, tag="pr")
                nc.gpsimd.tensor_tensor(
                    pr[0:64, :], sbfi[0:64, :], st[l][:, :], MUL
                )
                nc.vector.scalar_tensor_tensor(
                    pr[64:128, :], sbog[64:128, :], 0.5, sbfi[64:128, :],
                    SUB, MUL,
                )
                gt[l] = (sbog, pr)
            for l in layers:
                nc.tensor.matmul(
                    pss[l][:, 256:384], p2sb[:], gt[l][1][:],
                    start=True, stop=True,
                )
            for l in layers:
                ps = pss[l]
                sbog, _ = gt[l]
                nc.vector.tensor_copy(st[l][:, :], ps[0:64, 256:384])
                uu = upool.tile([64, BL], f32, tag="uu")
                nc.scalar.activation(
                    uu[:, :], ps[0:64, 256:384], SIG, bias=0.0, scale=4.0
                )
                nc.vector.scalar_tensor_tensor(
                    hh[l][0:64, :], uu[:, :], 0.5, sbog[0:64, :], SUB, MUL
                )
            if y_stage_ap is not None:
                pt = pst.tile([BL, 128], f32, tag="pt", bufs=2)
                nc.tensor.transpose(pt[:], hh[L - 1][:, :], ident[:])
                nc.scalar.copy(y_stage_ap, pt[:, 0:HD])

        # --- ramp-in slots 0..ramp-1 ---
        for s in range(ramp):
            ex_t = rstage.tile([64, BL], f32, tag="ex_ramp", bufs=2)
            nc.sync.dma_start(ex_t[:], exs_d[s])
            slot(list(range(min(s, L - 1) + 1)), ex_t[:], None)

        # --- steady slots in For_i ---
        with tc.For_i(
            0, nit, 1,
            hint_engines=(
                mybir.EngineType.Activation,
                mybir.EngineType.DVE,
                mybir.EngineType.PE,
            ),
        ) as it:
            exstage = stages.tile([64, u * BL], f32, tag="exstage", bufs=2)
            nc.sync.dma_start(exstage[:], exm_d[ds(it, 1), :, :])
            ystage = stages.tile([BL, u, HD], f32, tag="ystage", bufs=2)
            for j in range(u):
                slot(
                    list(range(L)),
                    exstage[:, j * BL : (j + 1) * BL],
                    ystage[:, j, :],
                )
            nc.sync.dma_start(y_d[ds(it, 1), :, :, :], ystage[:])

        # --- ramp-out slots T..T+3 (layers w+1..4), taps t=1020..1023 ---
        ytail = stages.tile([BL, ramp, HD], f32, tag="ytail", bufs=1)
        for w in range(ramp):
            slot(list(range(w + 1, L)), None, ytail[:, w, :])
        nc.sync.dma_start(y_d[nit, :, 0:ramp, :], ytail[:])

        # --- epilogue: gather + head ---
        tc.strict_bb_all_engine_barrier()
        g4 = const.tile([BL, HD], f32, tag="g4")
        nc.gpsimd.indirect_dma_start(
            out=g4[:],
            out_offset=None,
            in_=y_d[:].rearrange("a b c d -> (a b c) d"),
            in_offset=bass.IndirectOffsetOnAxis(ap=gidx_sb[:, 0:1], axis=0),
        )
        ptr = pst.tile([HD, BL], f32, tag="pt", bufs=2)
        nc.tensor.transpose(ptr[:], g4[:], ident[:])
        hsb = const.tile([HD, BL], f32, tag="hsb")
        nc.scalar.copy(hsb[:], ptr[:])
        po = pst.tile([BL, 1], f32, tag="pt", bufs=2)
        nc.tensor.matmul(po[:], hsb[:], headw_sb[:], start=True, stop=True)
        osb = const.tile([BL, 1], f32, tag="osb")
        nc.scalar.copy(osb[:], po[:])
        nc.sync.dma_start(out_d[:], osb[:])

    nc.compile()
    return nc


def _prep_host(x, lengths, emb, W_i, W_f, W_g, W_o, b_i, b_f, b_g, b_o,
               init_h, head_w, head_b, u, nit):
    """Build per-core input maps."""
    x = np.asarray(x, dtype=np.int64)
    lengths = np.asarray(lengths, dtype=np.int64)
    emb = np.asarray(emb, dtype=np.float32)
    ramp = RAMP
    t_total = ramp + u * nit

    wmm = np.empty((L, 2, 2, 64, 128), dtype=np.float32)
    biasv = np.empty((128, 2 * L), dtype=np.float32)
    for l in range(L):
        sx = 1.0 if l == 0 else 2.0
        a_fi = np.concatenate([W_f[l], W_i[l]], axis=0)          # [128, CAT]
        a_og = np.concatenate([W_o[l], 2.0 * W_g[l]], axis=0)
        for g, a in enumerate((a_fi, a_og)):
            wmm[l, g, 0] = (a[:, :HD] * sx).T.astype(np.float32)
            wmm[l, g, 1] = (a[:, HD:] * 2.0).T.astype(np.float32)
        biasv[:, 2 * l] = np.concatenate([b_f[l], b_i[l]])
        biasv[:, 2 * l + 1] = np.concatenate([b_o[l], 2.0 * b_g[l]])

    hs0_1 = (np.tanh(np.asarray(init_h, dtype=np.float32)) / 2.0)  # [L, HD]
    headw = (2.0 * np.asarray(head_w, dtype=np.float32)[0])[:, None]

    p2m = np.zeros((128, 128), dtype=np.float32)
    for jj2 in range(64):
        p2m[jj2, jj2] = 1.0
        p2m[64 + jj2, jj2] = 1.0
    permm = np.stack([np.eye(128, dtype=np.float32), p2m])

    ex_all = emb[x]  # [B, T_model, H] float32

    in_maps = []
    for c in range(NCORES):
        sl = slice(c * BL, (c + 1) * BL)
        ex_c = ex_all[sl].transpose(1, 2, 0).astype(np.float32)  # [T, H, BL]
        exs = np.ascontiguousarray(ex_c[:ramp])
        exm = np.ascontiguousarray(
            ex_c[ramp:t_total].reshape(nit, u, HD, BL)
            .transpose(0, 2, 1, 3)
            .reshape(nit, HD, u * BL)
        )
        hs0 = np.repeat(hs0_1[:, :, None], BL, axis=2).astype(np.float32)
        t_b = lengths[sl].astype(np.int64) - 1  # in [0, T-1]
        steady = t_b < nit * u
        ii = np.where(steady, t_b // u, nit)
        jj = np.where(steady, t_b % u, t_b - nit * u)
        rows = (ii * BL + np.arange(BL)) * u + jj
        in_maps.append(
            {
                "wmm": wmm,
                "biasv": biasv,
                "hs0": hs0,
                "exs": exs,
                "exm": exm,
                "gidx": rows.astype(np.int32)[:, None],
                "headw": headw,
                "permm": permm,
            }
        )
    return in_maps


def kernel(x, lengths, emb, W_i, W_f, W_g, W_o, b_i, b_f, b_g, b_o,
           init_h, head_w, head_b, _trace=False):
    from concourse.bass_utils import run_bass_kernel_spmd

    key = (U, NIT)
    if key not in _COMPILED:
        _COMPILED[key] = _build(U, NIT)
    nc = _COMPILED[key]

    in_maps = _prep_host(
        x, lengths, emb, W_i, W_f, W_g, W_o, b_i, b_f, b_g, b_o,
        init_h, head_w, head_b, U, NIT,
    )
    res = run_bass_kernel_spmd(nc, in_maps, list(range(NCORES)), trace=_trace)
    outs = [res.results[c]["out"][:, 0] for c in range(NCORES)]
    logits = np.concatenate(outs).astype(np.float32) + np.float32(
        np.asarray(head_b).reshape(-1)[0]
    )
    if _trace:
        kernel._last_exec_time_ns = res.exec_time_ns
        kernel._last_profile = res.profile_json
    return logits
